# revision 5
# baseline (speedup 1.0000x reference)
"""TRN2 Bass kernel for nn_ComboFwdVecComp (B=4, S=512, C=V=128).

out[b,i,j,v] = tanh( sum_c ctx[b,i,c]*ctx[b,j,c]*Wm[v,c]        (M term)
                     + ctx[b,j,:] @ (W1+Wd).T                    (A term, j-dep)
                     + ctx[b,i,:] @ (W2-Wd).T + (b1+b2+bm+bd)    (Brow, i-dep) )

Output (4,512,512,128) f32 = 512 MiB -> memory-bound (HBM write dominated).

Sharding: 8 cores, core k handles b = k//2, i in [ (k%2)*256, +256 ).
Each core emits out_shard (256, 512, 128) = 64 MiB; host concatenates.

Per-core algorithm (quad = 4 consecutive i, psum tile [128, 2048] = 4 banks,
free layout (jc, iq, v)):
  rhs'_q[c, (iq,v)] = WmT[c,v]*ctxi[c,i] + AW[c,v]      (DVE, fp32r-rounded)
  psum_q: per jc: bias mm (K=1, ones^T @ Brow_quad, start=True)
          then    main mm (K=128, ctxT_chunk_jc^T @ rhs'_q, N=512)
  out_sb = tanh(psum_q)                                  (ACT, [128,2048])
  DMA out_sb -> out_shard[4q:4q+4]                       (1 MiB per quad)

Matmuls run in float32r (TF32-like, ~1.6e-4 rel err, ~1 cyc/row at N=512;
plain fp32 is 4 cyc/row = PE-bound). fp32r inputs must be produced by a
rounding compute op, so constants are rounded on-chip via DVE copies.
brow rows must sit at partition base 0/32/64 for the K=1 matmul, so the
host packs them on partitions 0 (quads 0..31) and 32 (quads 32..63).
"""

import sys
import types
from contextlib import ExitStack

import numpy as np

import concourse.bass as bass
import concourse.mybir as mybir
import concourse.tile as tile
from concourse import bacc
from concourse.bass_utils import run_bass_kernel_spmd

B, S, C, V = 4, 512, 128, 128
NCORES = 8
NI = 256          # i's per core
NQ = NI // 4      # quads per core (64)
QHALF = NQ // 2   # quads per brow partition row (32)

_F32 = mybir.dt.float32
_F32R = mybir.dt.float32r


def install_ntff_shim():
    """antenv.axon_hooks is absent on some images; shim it so trace=True works."""
    if "antenv.axon_hooks" in sys.modules:
        return
    try:
        from trn_agent_boot.trn_boot import _ntff_profile_via_ctypes
        hook = _ntff_profile_via_ctypes("/opt/axon/libaxon_pjrt.so")
    except Exception:
        hook = None
    mod = types.ModuleType("antenv.axon_hooks")
    mod.get_axon_ntff_profile_hook = lambda: hook
    mod.set_axon_ntff_profile_hook = lambda h: None
    sys.modules["antenv.axon_hooks"] = mod


def build_nc():
    nc = bacc.Bacc("TRN2", target_bir_lowering=False, debug=False)

    ctxT_d = nc.dram_tensor("ctxT", [C, S], _F32, kind="ExternalInput").ap()
    ctxi_d = nc.dram_tensor("ctxi", [C, NI], _F32, kind="ExternalInput").ap()
    wmT_d = nc.dram_tensor("wmT", [C, V], _F32, kind="ExternalInput").ap()
    aw_d = nc.dram_tensor("aw", [C, V], _F32, kind="ExternalInput").ap()
    # brow rows: partition 0 holds quads 0..31, partition 32 quads 32..63
    browp_d = nc.dram_tensor("browp", [33, QHALF * 4 * V], _F32, kind="ExternalInput").ap()
    out_d = nc.dram_tensor("out_shard", [NI, S, V], _F32, kind="ExternalOutput").ap()

    with tile.TileContext(nc) as tc, ExitStack() as ctx:
        singles = ctx.enter_context(tc.tile_pool(name="singles", bufs=1))
        rhs_pool = ctx.enter_context(tc.tile_pool(name="rhs", bufs=3))
        tmp_pool = ctx.enter_context(tc.tile_pool(name="tmp", bufs=3))
        psum_pool = ctx.enter_context(tc.tile_pool(name="psum", bufs=2, space="PSUM"))
        out_pool = ctx.enter_context(tc.tile_pool(name="outs", bufs=3))

        # ---- load constants ----
        ctxT_sb = singles.tile([C, S], _F32)
        ctxi_sb = singles.tile([C, NI], _F32)
        wmT_sb = singles.tile([C, V], _F32)
        aw_sb = singles.tile([C, V], _F32)
        browp_sb = singles.tile([33, QHALF * 4 * V], _F32)
        nc.sync.dma_start(out=ctxT_sb, in_=ctxT_d)
        nc.sync.dma_start(out=ctxi_sb, in_=ctxi_d)
        nc.sync.dma_start(out=wmT_sb, in_=wmT_d)
        nc.sync.dma_start(out=aw_sb, in_=aw_d)
        nc.sync.dma_start(out=browp_sb, in_=browp_d)

        # ---- round fp32r operands on-chip (DVE copy = rounding producer) ----
        ctxT_r = singles.tile([C, S], _F32R)
        nc.vector.tensor_copy(ctxT_r, ctxT_sb)
        browp_r = singles.tile([33, QHALF * 4 * V], _F32R)
        # chunked so quad 0's bias mm doesn't wait on the full 16K-cycle copy
        RCH = QHALF * 4 * V // 4
        for cc in range(4):
            nc.vector.tensor_copy(
                browp_r[:, cc * RCH:(cc + 1) * RCH],
                browp_sb[:, cc * RCH:(cc + 1) * RCH],
            )
        ones_f = singles.tile([33, 128], _F32)
        nc.vector.memset(ones_f, 1.0)
        ones_r = singles.tile([33, 128], _F32R)
        nc.vector.tensor_copy(ones_r, ones_f)

        # aw broadcast x4 along a middle free dim for the quad-wide add
        aw_b4 = bass.AP(
            tensor=aw_sb.tensor,
            offset=aw_sb.offset,
            ap=[aw_sb.ap[0], [0, 4], aw_sb.ap[1]],
        )

        for q in range(NQ):
            # ---- DVE prep: rhs'_q = WmT * ctxi[:, i] (per-partition scalar) + AW
            tmp_q = tmp_pool.tile([C, 4 * V], _F32)
            for t in range(4):
                gi = 4 * q + t
                nc.vector.tensor_scalar(
                    tmp_q[:, t * V:(t + 1) * V],
                    wmT_sb,
                    ctxi_sb[:, gi: gi + 1],
                    None,
                    mybir.AluOpType.mult,
                )
            rhs_q = rhs_pool.tile([C, 4 * V], _F32R)
            nc.vector.tensor_tensor(
                out=rhs_q, in0=tmp_q, in1=aw_b4, op=mybir.AluOpType.add
            )

            # brow row + matching ones slice for this quad (base 0 or 32)
            base = 0 if q < QHALF else 32
            qq = q % QHALF
            brow_row = browp_r[base:base + 1, qq * 512:(qq + 1) * 512]
            ones_row = ones_r[base:base + 1, :]

            # ---- PE: bias mms then main mms into one 4-bank psum tile ----
            pt = psum_pool.tile([128, 2048], _F32, name="pq")
            for jc in range(4):
                nc.tensor.matmul(
                    pt[:, jc * 512:(jc + 1) * 512],
                    lhsT=ones_row,
                    rhs=brow_row,
                    start=True,
                    stop=False,
                )
            for jc in range(4):
                nc.tensor.matmul(
                    pt[:, jc * 512:(jc + 1) * 512],
                    lhsT=ctxT_r[:, jc * 128:(jc + 1) * 128],
                    rhs=rhs_q,
                    start=False,
                    stop=True,
                )

            # ---- ACT: tanh over the whole quad (4 psum banks) ----
            ot = out_pool.tile([128, 2048], _F32)
            nc.scalar.activation(ot, pt, mybir.ActivationFunctionType.Tanh)

            # ---- DMA out: (p=j_in, jc, iq, v) -> out[4q+iq, jc*128+j_in, v] ----
            for iq in range(4):
                dst = bass.AP(
                    tensor=out_d.tensor,
                    offset=(4 * q + iq) * S * V,
                    ap=[[V, 128], [128 * V, 4], [1, V]],
                )
                src = bass.AP(
                    tensor=ot.tensor,
                    offset=ot.offset + iq * V,
                    ap=[ot.ap[0], [512, 4], [1, V]],
                )
                nc.sync.dma_start(out=dst, in_=src)

    nc.compile()
    return nc


_NC_CACHE = {}


def get_nc():
    if "nc" not in _NC_CACHE:
        _NC_CACHE["nc"] = build_nc()
    return _NC_CACHE["nc"]


def make_in_maps(ctx, W1, b1, W2, b2, Wm, bm, Wd, bd):
    ctx = np.asarray(ctx, np.float32)
    bias_all = (
        np.asarray(b1) + np.asarray(b2) + np.asarray(bm) + np.asarray(bd)
    ).astype(np.float32)
    wmT = np.ascontiguousarray(np.asarray(Wm, np.float32).T)                  # (C,V)
    aw = np.ascontiguousarray(
        (np.asarray(W1) + np.asarray(Wd)).T.astype(np.float32)
    )
    w2d = (np.asarray(W2) - np.asarray(Wd)).astype(np.float32)                # (V,C)

    in_maps = []
    for k in range(NCORES):
        b = k // 2
        i0c = (k % 2) * NI
        brow = (ctx[b, i0c:i0c + NI] @ w2d.T + bias_all).astype(np.float32)   # (NI,V)
        browp = np.zeros((33, QHALF * 4 * V), np.float32)
        browp[0] = brow[: NI // 2].reshape(-1)
        browp[32] = brow[NI // 2:].reshape(-1)
        in_maps.append({
            "ctxT": np.ascontiguousarray(ctx[b].T),
            "ctxi": np.ascontiguousarray(ctx[b, i0c:i0c + NI].T),
            "wmT": wmT,
            "aw": aw,
            "browp": browp,
        })
    return in_maps


def run(in_maps, **kw):
    return run_bass_kernel_spmd(get_nc(), in_maps, core_ids=list(range(NCORES)), **kw)


def assemble(results):
    out = np.empty((B, S, S, V), np.float32)
    for k in range(NCORES):
        b = k // 2
        i0c = (k % 2) * NI
        out[b, i0c:i0c + NI] = results[k]["out_shard"]
    return out


def kernel(ctx, W1, b1, W2, b2, Wm, bm, Wd, bd):
    install_ntff_shim()
    in_maps = make_in_maps(ctx, W1, b1, W2, b2, Wm, bm, Wd, bd)
    res = run(in_maps)
    return assemble(res.results)
